# revision 2
# baseline (speedup 1.0000x reference)
"""Trainium2 Bass kernel for nn_ASVT_9500467658791 (ragged segment attention).

Pipeline (per point-cloud segment, one segment per NeuronCore, 8 cores):
  q/k/v = feat @ {Wq,Wk,Wv}  (1x1 convs)
  per-segment unscaled-softmax attention  r = softmax(q k^T) v
  t = r @ Wt ; BatchNorm over the full batch (training stats, synced across
  cores via a tiny AllGather) ; out = feat + relu(bn(t))

Layout strategy: everything d-major ("transposed") on chip.  The host
pre-transposes feat (bf16) so no on-device transposes of the input are
needed; q^T/k^T are produced 4x-replicated across partition bands so the
dqk=32 score matmuls can be packed 4-wide into the PE array with
tile_position.  Scores are computed keys-major ([key, query]) so the exp'd
attention matrix feeds the attn@v matmul directly as the stationary
operand; the ones-column appended to v yields the softmax denominators for
free.  r is re-transposed with the DMA xbar (SBUF->SBUF, bf16) for the
t = r @ Wt matmul; BN stats reduce along the free axis; the final
normalized result is xbar-transposed back and added to the f32 residual.
"""

import math
import os
from contextlib import ExitStack

import numpy as np
import ml_dtypes

import concourse.bass as bass
import concourse.bacc as bacc
import concourse.tile as tile
from concourse import mybir
from concourse import bass_utils

f32 = mybir.dt.float32
bf16 = mybir.dt.bfloat16
AF = mybir.ActivationFunctionType
ALU = mybir.AluOpType
AX = mybir.AxisListType

NCORES = 8
D = 256
DQK = 32
N_TOT = 16384
EPS = 1e-5
VW = 258          # v width: 256 d columns + ones column + pad
LP_MIN = 2304     # default segment pad (18 tiles); raised dynamically if needed

LAST_RESULT = None  # BassKernelResults of the most recent run (for test harness)
_NC_CACHE = {}


def _chunks(LP):
    out = []
    c0 = 0
    while c0 < LP:
        out.append((c0, min(512, LP - c0)))
        c0 += 512
    return out


def build_nc(LP):
    NT = LP // 128
    chunks = _chunks(LP)

    nc = bacc.Bacc("TRN2", target_bir_lowering=False, debug=False,
                   enable_asserts=True, num_devices=NCORES)

    featT_d = nc.dram_tensor("featT", [D, LP], bf16, kind="ExternalInput")
    featn_d = nc.dram_tensor("featn", [LP, D], f32, kind="ExternalInput")
    maskf_d = nc.dram_tensor("maskf", [LP, 1], f32, kind="ExternalInput")
    maskb_d = nc.dram_tensor("maskb", [LP, 1], bf16, kind="ExternalInput")
    wqr_d = nc.dram_tensor("wqr", [D, 128], bf16, kind="ExternalInput")
    wkr_d = nc.dram_tensor("wkr", [D, 128], bf16, kind="ExternalInput")
    wva_d = nc.dram_tensor("wva", [D, VW], bf16, kind="ExternalInput")
    wt_d = nc.dram_tensor("wt", [D, D], bf16, kind="ExternalInput")
    gamt_d = nc.dram_tensor("gamt", [D, 1], f32, kind="ExternalInput")
    bett_d = nc.dram_tensor("bett", [D, 1], f32, kind="ExternalInput")
    sel_d = nc.dram_tensor("sel", [4 * NCORES, 4], f32, kind="ExternalInput")
    out_d = nc.dram_tensor("out", [LP, D], f32, kind="ExternalOutput")

    cc_in = nc.dram_tensor("cc_in", [4, 128], f32, kind="Internal")
    cc_out = nc.dram_tensor("cc_out", [4 * NCORES, 128], f32, kind="Internal",
                            addr_space="Shared")

    with tile.TileContext(nc) as tc, ExitStack() as ctx:
        const = ctx.enter_context(tc.tile_pool(name="const", bufs=1))
        big = ctx.enter_context(tc.tile_pool(name="big", bufs=1))
        vpool = ctx.enter_context(tc.tile_pool(name="vpool", bufs=1))
        epool = ctx.enter_context(tc.tile_pool(name="epool", bufs=2))
        work = ctx.enter_context(tc.tile_pool(name="work", bufs=3))
        small = ctx.enter_context(tc.tile_pool(name="small", bufs=4))
        # PSUM: 8 banks total -> scores 5, accum(tT/misc) 1, r/v 2
        psS = ctx.enter_context(tc.tile_pool(name="psS", bufs=5, space="PSUM"))
        psA = ctx.enter_context(tc.tile_pool(name="psA", bufs=1, space="PSUM"))
        psR = ctx.enter_context(tc.tile_pool(name="psR", bufs=2, space="PSUM"))

        # ---------- constants / inputs ----------
        featT_sb = [big.tile([128, LP], bf16, tag=f"featT{h}", name=f"featT{h}") for h in range(2)]
        for h in range(2):
            nc.gpsimd.dma_start(out=featT_sb[h], in_=featT_d[128 * h:128 * (h + 1), :])

        featn_sb = big.tile([128, NT, D], f32, tag="featn")
        nc.gpsimd.dma_start(out=featn_sb,
                            in_=featn_d.rearrange("(n p) d -> p n d", p=128))
        maskf_sb = const.tile([128, NT], f32, tag="maskf")
        nc.gpsimd.dma_start(out=maskf_sb,
                            in_=maskf_d.rearrange("(n p) one -> p (n one)", p=128))
        maskb_sb = const.tile([128, NT], bf16, tag="maskb")
        nc.gpsimd.dma_start(out=maskb_sb,
                            in_=maskb_d.rearrange("(n p) one -> p (n one)", p=128))

        wqr_sb = [const.tile([128, 128], bf16, tag=f"wqr{h}", name=f"wqr{h}") for h in range(2)]
        wkr_sb = [const.tile([128, 128], bf16, tag=f"wkr{h}", name=f"wkr{h}") for h in range(2)]
        wva_sb = [const.tile([128, VW], bf16, tag=f"wva{h}", name=f"wva{h}") for h in range(2)]
        wt_sb = [const.tile([128, D], bf16, tag=f"wt{h}", name=f"wt{h}") for h in range(2)]
        for h in range(2):
            sl = slice(128 * h, 128 * (h + 1))
            nc.gpsimd.dma_start(out=wqr_sb[h], in_=wqr_d[sl, :])
            nc.gpsimd.dma_start(out=wkr_sb[h], in_=wkr_d[sl, :])
            nc.gpsimd.dma_start(out=wva_sb[h], in_=wva_d[sl, :])
            nc.gpsimd.dma_start(out=wt_sb[h], in_=wt_d[sl, :])
        gamt_sb = [const.tile([128, 1], f32, tag=f"gam{h}", name=f"gam{h}") for h in range(2)]
        bett_sb = [const.tile([128, 1], f32, tag=f"bet{h}", name=f"bet{h}") for h in range(2)]
        for h in range(2):
            sl = slice(128 * h, 128 * (h + 1))
            nc.gpsimd.dma_start(out=gamt_sb[h], in_=gamt_d[sl, :])
            nc.gpsimd.dma_start(out=bett_sb[h], in_=bett_d[sl, :])
        sel_sb = const.tile([4 * NCORES, 4], f32, tag="sel")
        nc.gpsimd.dma_start(out=sel_sb, in_=sel_d[:, :])

        # ---------- phase A: projections ----------
        # qT_rep / kT_rep [128, LP] bf16: each 32-row band holds the full
        # [32, LP] q^T / k^T (host replicated W 4x along columns).
        qT_sb = big.tile([128, LP], bf16, tag="qT")
        kT_sb = big.tile([128, LP], bf16, tag="kT")
        for (c0, cw) in chunks:
            csl = slice(c0, c0 + cw)
            for wrep, dst in ((wqr_sb, qT_sb), (wkr_sb, kT_sb)):
                ps = psS.tile([128, 512], f32, tag="s")
                nc.tensor.matmul(ps[:, :cw], lhsT=wrep[0], rhs=featT_sb[0][:, csl],
                                 start=True, stop=False)
                nc.tensor.matmul(ps[:, :cw], lhsT=wrep[1], rhs=featT_sb[1][:, csl],
                                 start=False, stop=True)
                nc.vector.tensor_copy(out=dst[:, csl], in_=ps[:, :cw])

        v_sb = []
        for i in range(NT):
            isl = slice(128 * i, 128 * (i + 1))
            ps = psR.tile([128, VW], f32, tag="r")
            nc.tensor.matmul(ps, lhsT=featT_sb[0][:, isl], rhs=wva_sb[0],
                             start=True, stop=False)
            nc.tensor.matmul(ps, lhsT=featT_sb[1][:, isl], rhs=wva_sb[1],
                             start=False, stop=True)
            vt = vpool.tile([128, VW], bf16, tag=f"v{i}", name=f"v{i}")
            nc.vector.tensor_copy(out=vt, in_=ps)
            # ones column = valid-key mask (zero for padded rows)
            nc.vector.tensor_copy(out=vt[:, 256:257], in_=maskb_sb[:, i:i + 1])
            v_sb.append(vt)

        # persistent outputs of the attention+transconv phase
        rT_sb = [big.tile([128, LP], bf16, tag=f"rT{h}", name=f"rT{h}") for h in range(2)]
        tT_sb = [big.tile([128, LP], f32, tag=f"tT{h}", name=f"tT{h}") for h in range(2)]
        nch = len(chunks)
        sums_t = [const.tile([128, nch], f32, tag=f"st{h}", name=f"st{h}") for h in range(2)]
        sums_q = [const.tile([128, nch], f32, tag=f"sq{h}", name=f"sq{h}") for h in range(2)]

        groups = [list(range(g, min(g + 4, NT))) for g in range(0, NT, 4)]

        # ---------- phase B/C/D: attention + r + tT, chunked over queries ----
        for ci, (c0, cw) in enumerate(chunks):
            csl = slice(c0, c0 + cw)
            # scores^T + exp: [key_tile, query_chunk]
            exp_t = []
            for _ in range(NT):
                exp_t.append(None)
            for grp in groups:
                for i, kt in enumerate(grp):
                    ps = psS.tile([128, 512], f32, tag="s")
                    ksl = slice(128 * kt, 128 * (kt + 1))
                    bsl = slice(32 * i, 32 * (i + 1))
                    nc.tensor.matmul(ps[:, :cw], lhsT=kT_sb[bsl, ksl],
                                     rhs=qT_sb[bsl, csl],
                                     start=True, stop=True,
                                     tile_position=(32 * i, 0))
                    et = epool.tile([128, 512], bf16, tag=f"e{kt}", name=f"e{kt}")
                    nc.scalar.activation(out=et[:, :cw], in_=ps[:, :cw], func=AF.Exp)
                    exp_t[kt] = et

            # attn @ v_aug -> r rows (query-major), rescale by 1/rowsum, and
            # xbar-transpose into rT
            for j in range(cw // 128):
                jj = c0 // 128 + j
                jsl = slice(128 * j, 128 * (j + 1))
                ps_r = psR.tile([128, VW], f32, tag="r")
                for kt in range(NT):
                    nc.tensor.matmul(ps_r, lhsT=exp_t[kt][:, jsl], rhs=v_sb[kt],
                                     start=(kt == 0), stop=(kt == NT - 1))
                rs = small.tile([128, 1], f32, tag="rs")
                nc.vector.tensor_scalar_max(out=rs, in0=ps_r[:, 256:257],
                                            scalar1=1e-30)
                rec = small.tile([128, 1], f32, tag="rec")
                nc.vector.reciprocal(out=rec, in_=rs)
                scl = small.tile([128, 1], f32, tag="scl")
                nc.vector.tensor_mul(out=scl, in0=rec, in1=maskf_sb[:, jj:jj + 1])
                r_sb = work.tile([128, 256], bf16, tag="r_sb")
                nc.vector.tensor_scalar_mul(out=r_sb, in0=ps_r[:, 0:256],
                                            scalar1=scl)
                for h in range(2):
                    nc.sync.dma_start_transpose(
                        out=rT_sb[h][:, 128 * jj:128 * (jj + 1)],
                        in_=r_sb[:, 128 * h:128 * (h + 1)])

            # tT = Wt^T @ rT for this query chunk + BN partial stats
            for h in range(2):
                hsl = slice(128 * h, 128 * (h + 1))
                ps_t = psA.tile([128, 512], f32, tag="a")
                nc.tensor.matmul(ps_t[:, :cw], lhsT=wt_sb[0][:, hsl],
                                 rhs=rT_sb[0][:, csl], start=True, stop=False)
                nc.tensor.matmul(ps_t[:, :cw], lhsT=wt_sb[1][:, hsl],
                                 rhs=rT_sb[1][:, csl], start=False, stop=True)
                nc.scalar.activation(out=tT_sb[h][:, csl], in_=ps_t[:, :cw],
                                     func=AF.Copy,
                                     accum_out=sums_t[h][:, ci:ci + 1])
                sq = work.tile([128, 512], f32, tag="sq")
                nc.vector.tensor_mul(out=sq[:, :cw], in0=tT_sb[h][:, csl],
                                     in1=tT_sb[h][:, csl])
                nc.vector.reduce_sum(out=sums_q[h][:, ci:ci + 1], in_=sq[:, :cw],
                                     axis=AX.X)

        # ---------- phase E: global BN stats via AllGather ----------
        stf = const.tile([128, 4], f32, tag="stf")
        for h in range(2):
            nc.vector.reduce_sum(out=stf[:, h:h + 1], in_=sums_t[h], axis=AX.X)
            nc.vector.reduce_sum(out=stf[:, 2 + h:3 + h], in_=sums_q[h], axis=AX.X)
        for j in range(4):
            nc.gpsimd.dma_start(out=cc_in[j:j + 1, :], in_=stf[:, j:j + 1])
        nc.gpsimd.collective_compute(
            "AllGather", ALU.bypass,
            replica_groups=[list(range(NCORES))],
            ins=[cc_in[:, :]], outs=[cc_out[:, :]])
        ag_sb = const.tile([4 * NCORES, 128], f32, tag="ag")
        nc.gpsimd.dma_start(out=ag_sb, in_=cc_out[:, :])
        ps_g = psR.tile([128, VW], f32, tag="r")
        nc.tensor.matmul(ps_g[:, 0:4], lhsT=ag_sb, rhs=sel_sb, start=True, stop=True)
        statsT = const.tile([128, 4], f32, tag="statsT")
        nc.vector.tensor_copy(out=statsT, in_=ps_g[:, 0:4])

        scale_h, bias_h = [], []
        inv_n = 1.0 / float(N_TOT)
        for h in range(2):
            mu = small.tile([128, 1], f32, tag="mu")
            nc.vector.tensor_scalar_mul(out=mu, in0=statsT[:, h:h + 1], scalar1=inv_n)
            varp = small.tile([128, 1], f32, tag="varp")
            # varp = E[t^2] - mu^2 + eps  via (msq*inv_n - mu*mu) + eps
            musq = small.tile([128, 1], f32, tag="musq")
            nc.vector.tensor_mul(out=musq, in0=mu, in1=mu)
            msq = small.tile([128, 1], f32, tag="msq")
            nc.vector.tensor_scalar(out=msq, in0=statsT[:, 2 + h:3 + h],
                                    scalar1=inv_n, scalar2=None, op0=ALU.mult)
            nc.vector.tensor_sub(out=varp, in0=msq, in1=musq)
            nc.vector.tensor_scalar_add(out=varp, in0=varp, scalar1=EPS)
            sd = small.tile([128, 1], f32, tag="sd")
            nc.scalar.activation(out=sd, in_=varp, func=AF.Sqrt)
            rsig = small.tile([128, 1], f32, tag="rsig")
            nc.vector.reciprocal(out=rsig, in_=sd)
            # one Newton step: rsig' = rsig * (1.5 - 0.5 * varp * rsig^2)
            t1 = small.tile([128, 1], f32, tag="t1")
            nc.vector.tensor_mul(out=t1, in0=rsig, in1=rsig)
            t2 = small.tile([128, 1], f32, tag="t2")
            nc.vector.tensor_mul(out=t2, in0=t1, in1=varp)
            nc.vector.tensor_scalar(out=t2, in0=t2, scalar1=-0.5, scalar2=1.5,
                                    op0=ALU.mult, op1=ALU.add)
            nc.vector.tensor_mul(out=rsig, in0=rsig, in1=t2)
            sc = small.tile([128, 1], f32, tag="sc")
            nc.vector.tensor_mul(out=sc, in0=rsig, in1=gamt_sb[h])
            bi = small.tile([128, 1], f32, tag="bi")
            nc.vector.tensor_mul(out=bi, in0=mu, in1=sc)
            nc.vector.tensor_sub(out=bi, in0=bett_sb[h], in1=bi)
            scale_h.append(sc)
            bias_h.append(bi)

        # ---------- phase F: BN apply + relu, transpose back, residual ------
        outT_sb = [big.tile([128, LP], bf16, tag=f"oT{h}", name=f"oT{h}") for h in range(2)]
        for (c0, cw) in chunks:
            csl = slice(c0, c0 + cw)
            for h in range(2):
                nc.scalar.activation(out=outT_sb[h][:, csl], in_=tT_sb[h][:, csl],
                                     func=AF.Relu, bias=bias_h[h], scale=scale_h[h])
        for j in range(NT):
            jsl = slice(128 * j, 128 * (j + 1))
            outn = work.tile([128, 256], bf16, tag="outn")
            for h in range(2):
                nc.sync.dma_start_transpose(out=outn[:, 128 * h:128 * (h + 1)],
                                            in_=outT_sb[h][:, jsl])
            res = work.tile([128, 256], f32, tag="res")
            nc.vector.tensor_add(out=res, in0=featn_sb[:, j, :], in1=outn)
            nc.gpsimd.dma_start(out=out_d[jsl, :], in_=res)

    nc.compile()
    return nc


def _get_nc(LP):
    if LP not in _NC_CACHE:
        _NC_CACHE[LP] = build_nc(LP)
    return _NC_CACHE[LP]


def kernel(**inputs):
    global LAST_RESULT
    feat = np.asarray(inputs["feat"], dtype=np.float32)
    bids = np.asarray(inputs["bids"])
    Wq = np.asarray(inputs["Wq"], dtype=np.float32)
    Wk = np.asarray(inputs["Wk"], dtype=np.float32)
    Wv = np.asarray(inputs["Wv"], dtype=np.float32)
    Wt = np.asarray(inputs["Wt"], dtype=np.float32)
    gamma = np.asarray(inputs["gamma"], dtype=np.float32)
    beta = np.asarray(inputs["beta"], dtype=np.float32)

    n, d = feat.shape
    assert d == D
    starts = np.searchsorted(bids, np.arange(NCORES)).astype(np.int64)
    ends = np.append(starts[1:], n)
    lens = (ends - starts).astype(np.int64)
    maxlen = int(lens.max())
    LP = max(LP_MIN, ((maxlen + 127) // 128) * 128)
    nc = _get_nc(LP)

    wqr = np.concatenate([Wq] * 4, axis=1).astype(ml_dtypes.bfloat16)
    wkr = np.concatenate([Wk] * 4, axis=1).astype(ml_dtypes.bfloat16)
    wva = np.zeros((D, VW), dtype=ml_dtypes.bfloat16)
    wva[:, :D] = Wv.astype(ml_dtypes.bfloat16)
    wt = Wt.astype(ml_dtypes.bfloat16)
    gamt = gamma.reshape(D, 1).copy()
    bett = beta.reshape(D, 1).copy()
    sel = np.zeros((4 * NCORES, 4), dtype=np.float32)
    for p in range(4 * NCORES):
        sel[p, p % 4] = 1.0

    in_maps = []
    for c in range(NCORES):
        seg = feat[starts[c]:ends[c]]
        L = seg.shape[0]
        featn = np.zeros((LP, D), dtype=np.float32)
        featn[:L] = seg
        featT = np.ascontiguousarray(featn.T).astype(ml_dtypes.bfloat16)
        maskf = np.zeros((LP, 1), dtype=np.float32)
        maskf[:L] = 1.0
        in_maps.append({
            "featT": featT, "featn": featn,
            "maskf": maskf, "maskb": maskf.astype(ml_dtypes.bfloat16),
            "wqr": wqr, "wkr": wkr, "wva": wva, "wt": wt,
            "gamt": gamt, "bett": bett, "sel": sel,
        })

    trace_cores = None
    if os.environ.get("BASS_TRACE"):
        trace_cores = list(range(NCORES))
    res = bass_utils.run_bass_kernel_spmd(
        nc, in_maps, core_ids=list(range(NCORES)), trace_cores=trace_cores)
    LAST_RESULT = res

    out = np.empty((n, D), dtype=np.float32)
    for c in range(NCORES):
        out[starts[c]:ends[c]] = res.results[c]["out"][:lens[c]]
    return out


# revision 3
# speedup vs baseline: 1.3791x; 1.3791x over previous
"""Trainium2 Bass kernel for nn_ASVT_9500467658791 (ragged segment attention).

Pipeline (per point-cloud segment, one segment per NeuronCore, 8 cores):
  q/k/v = feat @ {Wq,Wk,Wv}  (1x1 convs)
  per-segment unscaled-softmax attention  r = softmax(q k^T) v
  t = r @ Wt ; BatchNorm over the full batch (training stats, synced across
  cores via a tiny AllGather) ; out = feat + relu(bn(t))

Layout strategy: everything d-major ("transposed") on chip.  The host
pre-transposes feat so no on-device transposes of the input are needed;
q^T/k^T are produced 4x-replicated across partition bands so the dqk=32
score matmuls can be packed 4-wide into the PE array with tile_position.
The q/k/score path runs in float32r (TF32-like, full PE rate at N>=256).
Scores are computed keys-major ([key, query]) so the exp'd attention
matrix feeds the attn@v matmul directly as the stationary operand; the
mask-column appended to v yields the softmax denominators for free and
kills padded keys.  r is re-transposed with PE transpose-mode matmuls for
the t = r @ Wt matmul; BN stats reduce along the free axis; the final
output stays d-major (residual added from the exact f32 bits of featT)
and the host transposes it back during unsharding.
"""

import os
from contextlib import ExitStack

import numpy as np
import ml_dtypes

import concourse.bass as bass
import concourse.bacc as bacc
import concourse.tile as tile
from concourse import mybir
from concourse import bass_utils
from concourse.masks import make_identity

f32 = mybir.dt.float32
f32r = mybir.dt.float32r
bf16 = mybir.dt.bfloat16
AF = mybir.ActivationFunctionType
ALU = mybir.AluOpType
AX = mybir.AxisListType

NCORES = 8
D = 256
N_TOT = 16384
EPS = 1e-5
VW = 258          # v width: 256 d columns + mask column + pad
LP_MIN = 2176     # default segment pad (17 tiles); raised dynamically if needed

LAST_RESULT = None  # BassKernelResults of the most recent run (for test harness)
_NC_CACHE = {}


def _chunks(LP):
    out, c0 = [], 0
    while c0 < LP:
        out.append((c0, min(512, LP - c0)))
        c0 += 512
    return out


def build_nc(LP):
    NT = LP // 128
    chunks = _chunks(LP)

    nc = bacc.Bacc("TRN2", target_bir_lowering=False, debug=False,
                   enable_asserts=True, num_devices=NCORES)

    featT_d = nc.dram_tensor("featT", [D, LP], f32r, kind="ExternalInput")
    maskf_d = nc.dram_tensor("maskf", [LP, 1], f32, kind="ExternalInput")
    wqr_d = nc.dram_tensor("wqr", [D, 128], f32r, kind="ExternalInput")
    wkr_d = nc.dram_tensor("wkr", [D, 128], f32r, kind="ExternalInput")
    wva_d = nc.dram_tensor("wva", [D, VW], f32r, kind="ExternalInput")
    wt_d = nc.dram_tensor("wt", [D, D], bf16, kind="ExternalInput")
    gamt_d = nc.dram_tensor("gamt", [D, 1], f32, kind="ExternalInput")
    bett_d = nc.dram_tensor("bett", [D, 1], f32, kind="ExternalInput")
    sel_d = nc.dram_tensor("sel", [4 * NCORES, 4], f32, kind="ExternalInput")
    out_d = nc.dram_tensor("out", [D, LP], f32, kind="ExternalOutput")

    cc_in = nc.dram_tensor("cc_in", [4, 128], f32, kind="Internal")
    cc_out = nc.dram_tensor("cc_out", [4 * NCORES, 128], f32, kind="Internal",
                            addr_space="Shared")

    with tile.TileContext(nc) as tc, ExitStack() as ctx:
        const = ctx.enter_context(tc.tile_pool(name="const", bufs=1))
        big = ctx.enter_context(tc.tile_pool(name="big", bufs=1))
        vpool = ctx.enter_context(tc.tile_pool(name="vpool", bufs=1))
        epool = ctx.enter_context(tc.tile_pool(name="epool", bufs=2))
        work = ctx.enter_context(tc.tile_pool(name="work", bufs=3))
        small = ctx.enter_context(tc.tile_pool(name="small", bufs=4))
        # PSUM: 8 banks total = scores 4 + tT 1 + r/v 2 + transpose 1
        psS = ctx.enter_context(tc.tile_pool(name="psS", bufs=4, space="PSUM"))
        psA = ctx.enter_context(tc.tile_pool(name="psA", bufs=1, space="PSUM"))
        psR = ctx.enter_context(tc.tile_pool(name="psR", bufs=2, space="PSUM"))
        psT = ctx.enter_context(tc.tile_pool(name="psT", bufs=1, space="PSUM"))

        # ---------- constants / inputs ----------
        featT_sb = [big.tile([128, LP], f32r, tag=f"featT{h}", name=f"featT{h}")
                    for h in range(2)]
        for h in range(2):
            nc.sync.dma_start(out=featT_sb[h], in_=featT_d[128 * h:128 * (h + 1), :])

        maskf_sb = const.tile([128, NT], f32, tag="maskf")
        nc.sync.dma_start(out=maskf_sb,
                          in_=maskf_d.rearrange("(n p) one -> p (n one)", p=128))

        wqr_sb = [const.tile([128, 128], f32r, tag=f"wqr{h}", name=f"wqr{h}")
                  for h in range(2)]
        wkr_sb = [const.tile([128, 128], f32r, tag=f"wkr{h}", name=f"wkr{h}")
                  for h in range(2)]
        wva_sb = [const.tile([128, VW], f32r, tag=f"wva{h}", name=f"wva{h}")
                  for h in range(2)]
        wt_sb = [const.tile([128, D], bf16, tag=f"wt{h}", name=f"wt{h}")
                 for h in range(2)]
        for h in range(2):
            sl = slice(128 * h, 128 * (h + 1))
            nc.sync.dma_start(out=wqr_sb[h], in_=wqr_d[sl, :])
            nc.sync.dma_start(out=wkr_sb[h], in_=wkr_d[sl, :])
            nc.sync.dma_start(out=wva_sb[h], in_=wva_d[sl, :])
            nc.sync.dma_start(out=wt_sb[h], in_=wt_d[sl, :])
        gamt_sb = [const.tile([128, 1], f32, tag=f"gam{h}", name=f"gam{h}")
                   for h in range(2)]
        bett_sb = [const.tile([128, 1], f32, tag=f"bet{h}", name=f"bet{h}")
                   for h in range(2)]
        for h in range(2):
            sl = slice(128 * h, 128 * (h + 1))
            nc.sync.dma_start(out=gamt_sb[h], in_=gamt_d[sl, :])
            nc.sync.dma_start(out=bett_sb[h], in_=bett_d[sl, :])
        sel_sb = const.tile([4 * NCORES, 4], f32, tag="sel")
        nc.sync.dma_start(out=sel_sb, in_=sel_d[:, :])
        ident = const.tile([128, 128], bf16, tag="ident")
        make_identity(nc, ident)

        # ---------- phase A: projections ----------
        # qT_rep / kT_rep [128, LP]: each 32-row band holds the full
        # [32, LP] q^T / k^T (host replicated W 4x along columns).
        qT_sb = big.tile([128, LP], f32r, tag="qT")
        kT_sb = big.tile([128, LP], f32r, tag="kT")
        for (c0, cw) in chunks:
            csl = slice(c0, c0 + cw)
            for wrep, dst in ((wqr_sb, qT_sb), (wkr_sb, kT_sb)):
                ps = psS.tile([128, 512], f32, tag="s")
                nc.tensor.matmul(ps[:, :cw], lhsT=wrep[0], rhs=featT_sb[0][:, csl],
                                 start=True, stop=False)
                nc.tensor.matmul(ps[:, :cw], lhsT=wrep[1], rhs=featT_sb[1][:, csl],
                                 start=False, stop=True)
                nc.vector.tensor_copy(out=dst[:, csl], in_=ps[:, :cw])

        v_sb = []
        for i in range(NT):
            isl = slice(128 * i, 128 * (i + 1))
            ps = psR.tile([128, VW], f32, tag="r")
            nc.tensor.matmul(ps, lhsT=featT_sb[0][:, isl], rhs=wva_sb[0],
                             start=True, stop=False)
            nc.tensor.matmul(ps, lhsT=featT_sb[1][:, isl], rhs=wva_sb[1],
                             start=False, stop=True)
            vt = vpool.tile([128, VW], bf16, tag=f"v{i}", name=f"v{i}")
            nc.vector.tensor_copy(out=vt, in_=ps)
            # mask column: 1.0 for valid keys, 0.0 for padded rows
            nc.vector.tensor_copy(out=vt[:, 256:257], in_=maskf_sb[:, i:i + 1])
            v_sb.append(vt)

        rT_sb = [big.tile([128, LP], bf16, tag=f"rT{h}", name=f"rT{h}")
                 for h in range(2)]
        tT_sb = [big.tile([128, LP], f32, tag=f"tT{h}", name=f"tT{h}")
                 for h in range(2)]
        nch = len(chunks)
        sums_t = [const.tile([128, nch], f32, tag=f"st{h}", name=f"st{h}")
                  for h in range(2)]
        sums_q = [const.tile([128, nch], f32, tag=f"sq{h}", name=f"sq{h}")
                  for h in range(2)]

        groups = [list(range(g, min(g + 4, NT))) for g in range(0, NT, 4)]

        # ---------- phase B/C/D: attention + r + tT, chunked over queries ----
        for ci, (c0, cw) in enumerate(chunks):
            csl = slice(c0, c0 + cw)
            # scores^T + exp: [key_tile, query_chunk], 4-way row-packed
            exp_t = [None] * NT
            for grp in groups:
                for i, kt in enumerate(grp):
                    ps = psS.tile([128, 512], f32, tag="s")
                    ksl = slice(128 * kt, 128 * (kt + 1))
                    bsl = slice(32 * i, 32 * (i + 1))
                    nc.tensor.matmul(ps[:, :cw], lhsT=kT_sb[bsl, ksl],
                                     rhs=qT_sb[bsl, csl],
                                     start=True, stop=True,
                                     tile_position=(32 * i, 0))
                    et = epool.tile([128, 512], bf16, tag=f"e{kt}", name=f"e{kt}")
                    nc.scalar.activation(out=et[:, :cw], in_=ps[:, :cw], func=AF.Exp)
                    exp_t[kt] = et

            # attn @ v_aug -> r rows (query-major), rescale by 1/rowsum,
            # PE-transpose into rT
            for j in range(cw // 128):
                jj = c0 // 128 + j
                jsl = slice(128 * j, 128 * (j + 1))
                ps_r = psR.tile([128, VW], f32, tag="r")
                for kt in range(NT):
                    nc.tensor.matmul(ps_r, lhsT=exp_t[kt][:, jsl], rhs=v_sb[kt],
                                     start=(kt == 0), stop=(kt == NT - 1))
                rs = small.tile([128, 1], f32, tag="rs")
                nc.vector.tensor_scalar_max(out=rs, in0=ps_r[:, 256:257],
                                            scalar1=1e-30)
                rec = small.tile([128, 1], f32, tag="rec")
                nc.vector.reciprocal(out=rec, in_=rs)
                scl = small.tile([128, 1], f32, tag="scl")
                nc.vector.tensor_mul(out=scl, in0=rec, in1=maskf_sb[:, jj:jj + 1])
                r_sb = work.tile([128, 256], bf16, tag="r_sb")
                nc.vector.tensor_scalar_mul(out=r_sb, in0=ps_r[:, 0:256],
                                            scalar1=scl)
                for h in range(2):
                    ps_tr = psT.tile([128, 128], bf16, tag="tr")
                    nc.tensor.transpose(ps_tr, r_sb[:, 128 * h:128 * (h + 1)],
                                        ident)
                    nc.vector.tensor_copy(
                        out=rT_sb[h][:, 128 * jj:128 * (jj + 1)], in_=ps_tr)

            # tT = Wt^T @ rT for this query chunk + BN partial stats
            for h in range(2):
                hsl = slice(128 * h, 128 * (h + 1))
                ps_t = psA.tile([128, 512], f32, tag="a")
                nc.tensor.matmul(ps_t[:, :cw], lhsT=wt_sb[0][:, hsl],
                                 rhs=rT_sb[0][:, csl], start=True, stop=False)
                nc.tensor.matmul(ps_t[:, :cw], lhsT=wt_sb[1][:, hsl],
                                 rhs=rT_sb[1][:, csl], start=False, stop=True)
                nc.scalar.activation(out=tT_sb[h][:, csl], in_=ps_t[:, :cw],
                                     func=AF.Copy,
                                     accum_out=sums_t[h][:, ci:ci + 1])
                sq = work.tile([128, 512], f32, tag="sq")
                nc.vector.tensor_mul(out=sq[:, :cw], in0=tT_sb[h][:, csl],
                                     in1=tT_sb[h][:, csl])
                nc.vector.reduce_sum(out=sums_q[h][:, ci:ci + 1], in_=sq[:, :cw],
                                     axis=AX.X)

        # ---------- phase E: global BN stats via AllGather ----------
        stf = const.tile([128, 4], f32, tag="stf")
        for h in range(2):
            nc.vector.reduce_sum(out=stf[:, h:h + 1], in_=sums_t[h], axis=AX.X)
            nc.vector.reduce_sum(out=stf[:, 2 + h:3 + h], in_=sums_q[h], axis=AX.X)
        for j in range(4):
            nc.sync.dma_start(out=cc_in[j:j + 1, :], in_=stf[:, j:j + 1])
        nc.gpsimd.collective_compute(
            "AllGather", ALU.bypass,
            replica_groups=[list(range(NCORES))],
            ins=[cc_in[:, :]], outs=[cc_out[:, :]])
        ag_sb = const.tile([4 * NCORES, 128], f32, tag="ag")
        nc.sync.dma_start(out=ag_sb, in_=cc_out[:, :])
        ps_g = psR.tile([128, VW], f32, tag="r")
        nc.tensor.matmul(ps_g[:, 0:4], lhsT=ag_sb, rhs=sel_sb, start=True, stop=True)
        statsT = const.tile([128, 4], f32, tag="statsT")
        nc.vector.tensor_copy(out=statsT, in_=ps_g[:, 0:4])

        scale_h, bias_h = [], []
        inv_n = 1.0 / float(N_TOT)
        for h in range(2):
            mu = small.tile([128, 1], f32, tag="mu")
            nc.vector.tensor_scalar_mul(out=mu, in0=statsT[:, h:h + 1], scalar1=inv_n)
            musq = small.tile([128, 1], f32, tag="musq")
            nc.vector.tensor_mul(out=musq, in0=mu, in1=mu)
            msq = small.tile([128, 1], f32, tag="msq")
            nc.vector.tensor_scalar(out=msq, in0=statsT[:, 2 + h:3 + h],
                                    scalar1=inv_n, scalar2=None, op0=ALU.mult)
            varp = small.tile([128, 1], f32, tag="varp")
            nc.vector.tensor_sub(out=varp, in0=msq, in1=musq)
            nc.vector.tensor_scalar_add(out=varp, in0=varp, scalar1=EPS)
            sd = small.tile([128, 1], f32, tag="sd")
            nc.scalar.activation(out=sd, in_=varp, func=AF.Sqrt)
            rsig = small.tile([128, 1], f32, tag="rsig")
            nc.vector.reciprocal(out=rsig, in_=sd)
            # one Newton step: rsig' = rsig * (1.5 - 0.5 * varp * rsig^2)
            t1 = small.tile([128, 1], f32, tag="t1")
            nc.vector.tensor_mul(out=t1, in0=rsig, in1=rsig)
            t2 = small.tile([128, 1], f32, tag="t2")
            nc.vector.tensor_mul(out=t2, in0=t1, in1=varp)
            nc.vector.tensor_scalar(out=t2, in0=t2, scalar1=-0.5, scalar2=1.5,
                                    op0=ALU.mult, op1=ALU.add)
            nc.vector.tensor_mul(out=rsig, in0=rsig, in1=t2)
            sc = small.tile([128, 1], f32, tag="sc")
            nc.vector.tensor_mul(out=sc, in0=rsig, in1=gamt_sb[h])
            bi = small.tile([128, 1], f32, tag="bi")
            nc.vector.tensor_mul(out=bi, in0=mu, in1=sc)
            nc.vector.tensor_sub(out=bi, in0=bett_sb[h], in1=bi)
            scale_h.append(sc)
            bias_h.append(bi)

        # ---------- phase F: BN apply + relu + residual (stays d-major) -----
        for (c0, cw) in chunks:
            csl = slice(c0, c0 + cw)
            for h in range(2):
                relu_sb = work.tile([128, 512], f32, tag="relu")
                nc.scalar.activation(out=relu_sb[:, :cw], in_=tT_sb[h][:, csl],
                                     func=AF.Relu, bias=bias_h[h],
                                     scale=scale_h[h])
                o_sb = work.tile([128, 512], f32, tag="o_sb")
                nc.vector.tensor_add(out=o_sb[:, :cw], in0=relu_sb[:, :cw],
                                     in1=featT_sb[h].bitcast(f32)[:, csl])
                nc.sync.dma_start(out=out_d[128 * h:128 * (h + 1), csl],
                                  in_=o_sb[:, :cw])

    nc.compile()
    return nc


def _get_nc(LP):
    if LP not in _NC_CACHE:
        _NC_CACHE[LP] = build_nc(LP)
    return _NC_CACHE[LP]


def kernel(**inputs):
    global LAST_RESULT
    feat = np.asarray(inputs["feat"], dtype=np.float32)
    bids = np.asarray(inputs["bids"])
    Wq = np.asarray(inputs["Wq"], dtype=np.float32)
    Wk = np.asarray(inputs["Wk"], dtype=np.float32)
    Wv = np.asarray(inputs["Wv"], dtype=np.float32)
    Wt = np.asarray(inputs["Wt"], dtype=np.float32)
    gamma = np.asarray(inputs["gamma"], dtype=np.float32)
    beta = np.asarray(inputs["beta"], dtype=np.float32)

    n, d = feat.shape
    assert d == D
    starts = np.searchsorted(bids, np.arange(NCORES)).astype(np.int64)
    ends = np.append(starts[1:], n)
    lens = (ends - starts).astype(np.int64)
    maxlen = int(lens.max())
    LP = max(LP_MIN, ((maxlen + 127) // 128) * 128)
    nc = _get_nc(LP)

    wqr = np.ascontiguousarray(np.concatenate([Wq] * 4, axis=1))
    wkr = np.ascontiguousarray(np.concatenate([Wk] * 4, axis=1))
    wva = np.zeros((D, VW), dtype=np.float32)
    wva[:, :D] = Wv
    wt = Wt.astype(ml_dtypes.bfloat16)
    gamt = gamma.reshape(D, 1).copy()
    bett = beta.reshape(D, 1).copy()
    sel = np.zeros((4 * NCORES, 4), dtype=np.float32)
    for p in range(4 * NCORES):
        sel[p, p % 4] = 1.0

    in_maps = []
    for c in range(NCORES):
        seg = feat[starts[c]:ends[c]]
        L = seg.shape[0]
        featT = np.zeros((D, LP), dtype=np.float32)
        featT[:, :L] = seg.T
        maskf = np.zeros((LP, 1), dtype=np.float32)
        maskf[:L] = 1.0
        in_maps.append({
            "featT": featT, "maskf": maskf,
            "wqr": wqr, "wkr": wkr, "wva": wva, "wt": wt,
            "gamt": gamt, "bett": bett, "sel": sel,
        })

    trace_cores = None
    if os.environ.get("BASS_TRACE"):
        trace_cores = list(range(NCORES))
    res = bass_utils.run_bass_kernel_spmd(
        nc, in_maps, core_ids=list(range(NCORES)), trace_cores=trace_cores)
    LAST_RESULT = res

    out = np.empty((n, D), dtype=np.float32)
    for c in range(NCORES):
        out[starts[c]:ends[c]] = res.results[c]["out"].T[:lens[c]]
    return out


# revision 20
# speedup vs baseline: 1.5130x; 1.0971x over previous
"""Trainium2 Bass kernel for nn_ASVT_9500467658791 (ragged segment attention).

Pipeline (per point-cloud segment, one segment per NeuronCore, 8 cores):
  q/k/v = feat @ {Wq,Wk,Wv}  (1x1 convs)
  per-segment unscaled-softmax attention  r = softmax(q k^T) v
  t = r @ Wt ; BatchNorm over the full batch (training stats, synced across
  cores via a tiny AllGather) ; out = feat + relu(bn(t))

Layout strategy: everything d-major ("transposed") on chip.  The host
pre-transposes feat so no on-device transposes of the input are needed;
q^T/k^T are produced 4x-replicated across partition bands so the dqk=32
score matmuls can be packed 4-wide into the PE array with tile_position.
The q/k/score path runs in float32r (TF32-like).  Scores are computed
keys-major ([key, query]) and exp'd in kt-pair batches; the attention
matrix then streams as the MOVING operand against stationary v-halves,
accumulating r^T [d, q] directly (no transposes anywhere).  Softmax
denominators come from an extra mask-column matmul per key tile
([1, q] accumulator), inverted once per chunk and partition-broadcast.
BN stats reduce along the free axis of t^T; the final output stays
d-major (residual added from the exact f32 bits of featT) and the host
transposes it back during unsharding.
"""

import os
from contextlib import ExitStack

import numpy as np
import ml_dtypes

import concourse.bass as bass
import concourse.bacc as bacc
import concourse.tile as tile
from concourse import mybir
from concourse import bass_utils
from concourse.masks import make_identity

f32 = mybir.dt.float32
f32r = mybir.dt.float32r
bf16 = mybir.dt.bfloat16
AF = mybir.ActivationFunctionType
ALU = mybir.AluOpType
AX = mybir.AxisListType

NCORES = 8
D = 256
N_TOT = 16384
EPS = 1e-5
LP_MIN = 2176     # default segment pad (17 tiles); raised dynamically if needed
WARMUP_CC = bool(int(__import__("os").environ.get("WARMUP_CC", "1")))
WARMUP_MM = bool(int(__import__("os").environ.get("WARMUP_MM", "1")))
EXPPAIR = bool(int(__import__("os").environ.get("EXPPAIR", "1")))
STATS_TR = bool(int(__import__("os").environ.get("STATS_TR", "1")))
TRUNC = int(__import__("os").environ.get("TRUNC", "0"))
SINGLE_BANK_S = bool(int(__import__("os").environ.get("SINGLE_BANK_S", "1")))

LAST_RESULT = None  # BassKernelResults of the most recent run (for test harness)
_NC_CACHE = {}


def _chunks(LP):
    out, c0 = [], 0
    while c0 < LP:
        out.append((c0, min(512, LP - c0)))
        c0 += 512
    return out


def build_nc(LP):
    NT = LP // 128
    chunks = _chunks(LP)

    nc = bacc.Bacc("TRN2", target_bir_lowering=False, debug=False,
                   enable_asserts=True, num_devices=NCORES)

    featT_d = nc.dram_tensor("featT", [D, LP], f32r, kind="ExternalInput")
    maskf_d = nc.dram_tensor("maskf", [LP, 1], f32, kind="ExternalInput")
    maskr_d = nc.dram_tensor("maskr", [1, LP], f32, kind="ExternalInput")
    wqr_d = nc.dram_tensor("wqr", [D, 128], f32r, kind="ExternalInput")
    wkr_d = nc.dram_tensor("wkr", [D, 128], f32r, kind="ExternalInput")
    wv_d = nc.dram_tensor("wv", [D, D], f32r, kind="ExternalInput")
    wt_d = nc.dram_tensor("wt", [D, D], bf16, kind="ExternalInput")
    gamt_d = nc.dram_tensor("gamt", [D, 1], f32, kind="ExternalInput")
    bett_d = nc.dram_tensor("bett", [D, 1], f32, kind="ExternalInput")
    sel_d = nc.dram_tensor("sel", [4 * NCORES, 4], f32, kind="ExternalInput")
    out_d = nc.dram_tensor("out", [D, LP], f32, kind="ExternalOutput")

    cc_in = nc.dram_tensor("cc_in", [4, 128], f32, kind="Internal")
    cc_out = nc.dram_tensor("cc_out", [4 * NCORES, 128], f32, kind="Internal",
                            addr_space="Shared")
    cc_in2 = nc.dram_tensor("cc_in2", [4, 128], f32, kind="Internal")
    cc_out2 = nc.dram_tensor("cc_out2", [4 * NCORES, 128], f32, kind="Internal",
                             addr_space="Shared")
    dnd = nc.dram_tensor("dnd", [len(_chunks(LP)), 512], f32, kind="Internal")
    ccw_in = nc.dram_tensor("ccw_in", [1, 128], f32, kind="Internal")
    ccw_out = nc.dram_tensor("ccw_out", [NCORES, 128], f32, kind="Internal",
                             addr_space="Shared")

    with tile.TileContext(nc) as tc, ExitStack() as ctx:
        const = ctx.enter_context(tc.tile_pool(name="const", bufs=1))
        big = ctx.enter_context(tc.tile_pool(name="big", bufs=1))
        vpool = ctx.enter_context(tc.tile_pool(name="vpool", bufs=1))
        epool = ctx.enter_context(tc.tile_pool(name="epool", bufs=2))
        work = ctx.enter_context(tc.tile_pool(name="work", bufs=3))
        small = ctx.enter_context(tc.tile_pool(name="small", bufs=4))
        # PSUM: 8 banks = score-pairs 2x2 + rT accum 2 + denom 1 + tT 1
        psS = ctx.enter_context(tc.tile_pool(
            name="psS", bufs=(4 if SINGLE_BANK_S else 2), space="PSUM"))
        psV = ctx.enter_context(tc.tile_pool(name="psV", bufs=1, space="PSUM"))
        psD = ctx.enter_context(tc.tile_pool(name="psD", bufs=1, space="PSUM"))
        psA = ctx.enter_context(tc.tile_pool(name="psA", bufs=1, space="PSUM"))

        # ---------- PE clock warm-up: dense junk matmuls during input DMA ----
        ident = const.tile([128, 128], f32, tag="ident")
        make_identity(nc, ident)
        ident_b = const.tile([128, 128], bf16, tag="ident_b")
        nc.vector.tensor_copy(out=ident_b, in_=ident)
        if WARMUP_MM:
            ps_w = psA.tile([128, 512], f32, tag="a")
            for i in range(100):
                nc.tensor.matmul(ps_w[:, 0:128], lhsT=ident_b, rhs=ident_b,
                                 start=True, stop=True)
            warm_junk = const.tile([128, 1], f32, tag="warm_junk")
            nc.vector.tensor_copy(out=warm_junk, in_=ps_w[:, 0:1])

        # ---------- warm-up collective (runs on TOPSP during phase A) -------
        if WARMUP_CC:
            wz = const.tile([1, 128], f32, tag="wz")
            nc.vector.memset(wz, 0.0)
            nc.sync.dma_start(out=ccw_in[:, :], in_=wz)
            nc.gpsimd.collective_compute(
                "AllGather", ALU.bypass, replica_groups=[list(range(NCORES))],
                ins=[ccw_in[:, :]], outs=[ccw_out[:, :]])

        # ---------- constants / inputs ----------
        featT_sb = [big.tile([128, LP], f32r, tag=f"featT{h}", name=f"featT{h}")
                    for h in range(2)]
        for h in range(2):
            for (c0, cw) in chunks:
                nc.sync.dma_start(out=featT_sb[h][:, c0:c0 + cw],
                                  in_=featT_d[128 * h:128 * (h + 1), c0:c0 + cw])

        maskf_sb = const.tile([128, NT], f32, tag="maskf")
        nc.sync.dma_start(out=maskf_sb,
                          in_=maskf_d.rearrange("(n p) one -> p (n one)", p=128))
        maskb_sb = const.tile([128, NT], bf16, tag="maskb")
        nc.vector.tensor_copy(out=maskb_sb, in_=maskf_sb)
        maskbc_sb = const.tile([128, LP], f32, tag="maskbc")
        _mr = maskr_d[0:1, :]
        nc.sync.dma_start(out=maskbc_sb, in_=bass.AP(
            tensor=_mr.tensor, offset=_mr.offset, ap=[[0, 128]] + list(_mr.ap[1:])))
        ones128 = const.tile([128, 128], bf16, tag="ones128")
        nc.vector.memset(ones128, 1.0)
        msk128 = const.tile([128, 128], bf16, tag="msk128")
        _mb = maskb_sb[:, NT - 1:NT]
        nc.vector.tensor_copy(out=msk128, in_=bass.AP(
            tensor=_mb.tensor, offset=_mb.offset, ap=[list(_mb.ap[0]), [0, 128]]))

        wqr_sb = [const.tile([128, 128], f32r, tag=f"wqr{h}", name=f"wqr{h}")
                  for h in range(2)]
        wkr_sb = [const.tile([128, 128], f32r, tag=f"wkr{h}", name=f"wkr{h}")
                  for h in range(2)]
        wv_sb = [const.tile([128, D], f32r, tag=f"wv{h}", name=f"wv{h}")
                 for h in range(2)]
        wt_sb = [const.tile([128, D], bf16, tag=f"wt{h}", name=f"wt{h}")
                 for h in range(2)]
        for h in range(2):
            sl = slice(128 * h, 128 * (h + 1))
            nc.sync.dma_start(out=wqr_sb[h], in_=wqr_d[sl, :])
            nc.sync.dma_start(out=wkr_sb[h], in_=wkr_d[sl, :])
            nc.sync.dma_start(out=wv_sb[h], in_=wv_d[sl, :])
            nc.sync.dma_start(out=wt_sb[h], in_=wt_d[sl, :])
        gamt_sb = [const.tile([128, 1], f32, tag=f"gam{h}", name=f"gam{h}")
                   for h in range(2)]
        bett_sb = [const.tile([128, 1], f32, tag=f"bet{h}", name=f"bet{h}")
                   for h in range(2)]
        for h in range(2):
            sl = slice(128 * h, 128 * (h + 1))
            nc.sync.dma_start(out=gamt_sb[h], in_=gamt_d[sl, :])
            nc.sync.dma_start(out=bett_sb[h], in_=bett_d[sl, :])
        sel_sb = const.tile([4 * NCORES, 4], f32, tag="sel")
        nc.sync.dma_start(out=sel_sb, in_=sel_d[:, :])

        # ---------- phase A: projections ----------
        # qT_rep / kT_rep [128, LP]: each 32-row band holds the full
        # [32, LP] q^T / k^T (host replicated W 4x along columns).
        qT_sb = big.tile([128, LP], f32r, tag="qT")
        kT_sb = big.tile([128, LP], f32r, tag="kT")
        for (c0, cw) in chunks:
            csl = slice(c0, c0 + cw)
            for wrep, dst in ((wqr_sb, qT_sb), (wkr_sb, kT_sb)):
                ps = psS.tile([128, 512 if SINGLE_BANK_S else 1024], f32, tag="s")
                nc.tensor.matmul(ps[:, :cw], lhsT=wrep[0], rhs=featT_sb[0][:, csl],
                                 start=True, stop=False)
                nc.tensor.matmul(ps[:, :cw], lhsT=wrep[1], rhs=featT_sb[1][:, csl],
                                 start=False, stop=True)
                nc.vector.tensor_copy(out=dst[:, csl], in_=ps[:, :cw])

        v_sb = []
        for i in range(NT):
            isl = slice(128 * i, 128 * (i + 1))
            ps = psS.tile([128, 512], f32, tag="s", name=f"psv_proj{i}")
            nc.tensor.matmul(ps[:, 0:D], lhsT=featT_sb[0][:, isl], rhs=wv_sb[0],
                             start=True, stop=False)
            nc.tensor.matmul(ps[:, 0:D], lhsT=featT_sb[1][:, isl], rhs=wv_sb[1],
                             start=False, stop=True)
            vt = vpool.tile([128, D], bf16, tag=f"v{i}", name=f"v{i}")
            nc.vector.tensor_copy(out=vt, in_=ps[:, 0:D])
            v_sb.append(vt)

        rT_sb = [big.tile([128, LP], bf16, tag=f"rT{h}", name=f"rT{h}")
                 for h in range(2)]
        tT_sb = [big.tile([128, LP], f32, tag=f"tT{h}", name=f"tT{h}")
                 for h in range(2)]
        nch = len(chunks)
        sums_t = [const.tile([128, nch], f32, tag=f"st{h}", name=f"st{h}")
                  for h in range(2)]
        sums_q = [const.tile([128, nch], f32, tag=f"sq{h}", name=f"sq{h}")
                  for h in range(2)]

        # ---------- phases B-D: attention + r^T + t^T, chunked over queries --
        for ci, (c0, cw) in (enumerate(chunks) if TRUNC != 2 else []):
            csl = slice(c0, c0 + cw)
            pairs = [(kt, kt + 1 if kt + 1 < NT else None)
                     for kt in range(0, NT, 2)]

            ps_rt = None if TRUNC == 4 else [
                psV.tile([128, 512], f32, tag=f"v{h}", name=f"psv{h}")
                for h in range(2)]
            ps_d = None if TRUNC == 4 else psD.tile([128, 512], f32, tag="d")

            exp_of = {}
            for pi, (ka, kb) in enumerate(pairs):
                if SINGLE_BANK_S:
                    pss = [psS.tile([128, 512], f32, tag="s", name=f"pss{s_}")
                           for s_ in range(2 if kb is not None else 1)]
                else:
                    ps = psS.tile([128, 1024], f32, tag="s")
                for sub, kt in enumerate([ka] + ([kb] if kb is not None else [])):
                    i = (2 * pi + sub) % 4
                    ksl = slice(128 * kt, 128 * (kt + 1))
                    bsl = slice(32 * i, 32 * (i + 1))
                    dst_ap = (pss[sub][:, :cw] if SINGLE_BANK_S
                              else ps[:, sub * cw:sub * cw + cw])
                    nc.tensor.matmul(dst_ap,
                                     lhsT=kT_sb[bsl, ksl], rhs=qT_sb[bsl, csl],
                                     start=True, stop=True,
                                     tile_position=(32 * i, 0))
                et = epool.tile([128, 1024], bf16, tag=f"e{pi}", name=f"e{pi}")
                nsub = 1 if kb is None else 2
                if EXPPAIR and not SINGLE_BANK_S:
                    ew = cw if kb is None else 2 * cw
                    nc.scalar.activation(out=et[:, :ew], in_=ps[:, :ew], func=AF.Exp)
                else:
                    for s_ in range(nsub):
                        src_ap = (pss[s_][:, :cw] if SINGLE_BANK_S
                                  else ps[:, s_ * cw:s_ * cw + cw])
                        nc.scalar.activation(out=et[:, s_ * cw:s_ * cw + cw],
                                             in_=src_ap, func=AF.Exp)
                for sub, kt in enumerate([ka] + ([kb] if kb is not None else [])):
                    exp_of[kt] = (et, sub)

                # attn@v for the pair right away: rT += v[kt]^T-stationary MMs
                for sub, kt in ([] if TRUNC == 4 else
                                list(enumerate([ka] + ([kb] if kb is not None else [])))):
                    et_, s_ = exp_of[kt]
                    esl = slice(s_ * cw, s_ * cw + cw)
                    first = kt == 0
                    last = kt == NT - 1
                    for h in range(2):
                        nc.tensor.matmul(
                            ps_rt[h][:, :cw], lhsT=v_sb[kt][:, 128 * h:128 * (h + 1)],
                            rhs=et_[:, esl], start=first, stop=last)
                    nc.tensor.matmul(ps_d[:, :cw],
                                     lhsT=(msk128 if kt == NT - 1 else ones128),
                                     rhs=et_[:, esl], start=first, stop=last)

            if TRUNC == 4:
                for h in range(2):
                    ej = exp_of[NT - 1][0]
                    nc.vector.tensor_copy(out=rT_sb[h][:, csl],
                                          in_=ej[:, 0:cw])
            if TRUNC == 3:
                for h in range(2):
                    nc.vector.tensor_copy(out=rT_sb[h][:, csl],
                                          in_=ps_rt[h][:, :cw])
                djunk = small.tile([128, 1], f32, tag="djunk")
                nc.vector.tensor_copy(out=djunk, in_=ps_d[:, 0:1])
            # softmax denominators -> masked reciprocal (rows identical)
            if TRUNC not in (3, 4):
                dnf = work.tile([128, 512], f32, tag="dnf")
                nc.vector.tensor_scalar_max(out=dnf[:, :cw], in0=ps_d[:, :cw],
                                            scalar1=1e-30)
                rec = work.tile([128, 512], f32, tag="recd")
                nc.vector.reciprocal(out=rec[:, :cw], in_=dnf[:, :cw])
                nc.vector.tensor_mul(out=rec[:, :cw], in0=rec[:, :cw],
                                     in1=maskbc_sb[:, csl])
                for h in range(2):
                    nc.vector.tensor_mul(out=rT_sb[h][:, csl],
                                         in0=ps_rt[h][:, :cw],
                                         in1=rec[:, :cw])

            # tT = Wt^T @ rT for this query chunk + BN partial stats
            for h in range(2):
                hsl = slice(128 * h, 128 * (h + 1))
                ps_t = psA.tile([128, 512], f32, tag="a")
                nc.tensor.matmul(ps_t[:, :cw], lhsT=wt_sb[0][:, hsl],
                                 rhs=rT_sb[0][:, csl], start=True, stop=False)
                nc.tensor.matmul(ps_t[:, :cw], lhsT=wt_sb[1][:, hsl],
                                 rhs=rT_sb[1][:, csl], start=False, stop=True)
                nc.scalar.activation(out=tT_sb[h][:, csl], in_=ps_t[:, :cw],
                                     func=AF.Copy,
                                     accum_out=sums_t[h][:, ci:ci + 1])
                sq = work.tile([128, 512], f32, tag="sq")
                nc.vector.tensor_mul(out=sq[:, :cw], in0=tT_sb[h][:, csl],
                                     in1=tT_sb[h][:, csl])
                nc.vector.reduce_sum(out=sums_q[h][:, ci:ci + 1], in_=sq[:, :cw],
                                     axis=AX.X)

            if ci == nch - 2 and not TRUNC:
                # early partial-stats AllGather (chunks 0..nch-2): overlaps the
                # last chunk's compute and re-syncs core skew before AG2
                stf1 = const.tile([128, 4], f32, tag="stf1")
                for h in range(2):
                    nc.vector.reduce_sum(out=stf1[:, h:h + 1],
                                         in_=sums_t[h][:, 0:nch - 1], axis=AX.X)
                    nc.vector.reduce_sum(out=stf1[:, 2 + h:3 + h],
                                         in_=sums_q[h][:, 0:nch - 1], axis=AX.X)
                ps_st1 = psD.tile([4, 128], f32, tag="d", name="ps_st1")
                nc.tensor.transpose(ps_st1, stf1, ident)
                stp1 = const.tile([4, 128], f32, tag="stp1")
                nc.vector.tensor_copy(out=stp1, in_=ps_st1)
                nc.sync.dma_start(out=cc_in[:, :], in_=stp1)
                nc.gpsimd.collective_compute(
                    "AllGather", ALU.bypass,
                    replica_groups=[list(range(NCORES))],
                    ins=[cc_in[:, :]], outs=[cc_out[:, :]])

        if TRUNC in (1, 3, 4):
            for h in range(2):
                nc.sync.dma_start(out=out_d[128 * h:128 * (h + 1), :], in_=tT_sb[h])
        elif TRUNC >= 2:
            for h, src_t in ((0, qT_sb), (1, kT_sb)):
                nc.sync.dma_start(out=out_d[128 * h:128 * (h + 1), :],
                                  in_=src_t.bitcast(f32))
        # ---------- phase E: global BN stats via AllGather ----------
        stf = None if TRUNC else const.tile([128, 4], f32, tag="stf")
        for h in range(2 if not TRUNC else 0):
            nc.vector.tensor_copy(out=stf[:, h:h + 1],
                                  in_=sums_t[h][:, nch - 1:nch])
            nc.vector.tensor_copy(out=stf[:, 2 + h:3 + h],
                                  in_=sums_q[h][:, nch - 1:nch])
        if not TRUNC:
            ps_st = psD.tile([4, 128], f32, tag="d", name="ps_st")
            nc.tensor.transpose(ps_st, stf, ident)
            stp = const.tile([4, 128], f32, tag="stp")
            nc.vector.tensor_copy(out=stp, in_=ps_st)
            nc.sync.dma_start(out=cc_in2[:, :], in_=stp)
        if not TRUNC:
            nc.gpsimd.collective_compute(
                "AllGather", ALU.bypass,
                replica_groups=[list(range(NCORES))],
                ins=[cc_in2[:, :]], outs=[cc_out2[:, :]])
            ag_sb = const.tile([4 * NCORES, 128], f32, tag="ag")
            nc.sync.dma_start(out=ag_sb, in_=cc_out[:, :])
            ag2_sb = const.tile([4 * NCORES, 128], f32, tag="ag2")
            nc.sync.dma_start(out=ag2_sb, in_=cc_out2[:, :])
            ps_g = psA.tile([128, 512], f32, tag="a", name="ps_g")
            nc.tensor.matmul(ps_g[:, 0:4], lhsT=ag_sb, rhs=sel_sb,
                             start=True, stop=False)
            nc.tensor.matmul(ps_g[:, 0:4], lhsT=ag2_sb, rhs=sel_sb,
                             start=False, stop=True)
            statsT = const.tile([128, 4], f32, tag="statsT")
            nc.vector.tensor_copy(out=statsT, in_=ps_g[:, 0:4])

        scale_h, bias_h = [], []
        inv_n = 1.0 / float(N_TOT)
        for h in range(2 if not TRUNC else 0):
            mu = small.tile([128, 1], f32, tag="mu")
            nc.vector.tensor_scalar_mul(out=mu, in0=statsT[:, h:h + 1], scalar1=inv_n)
            musq = small.tile([128, 1], f32, tag="musq")
            nc.vector.tensor_mul(out=musq, in0=mu, in1=mu)
            msq = small.tile([128, 1], f32, tag="msq")
            nc.vector.tensor_scalar(out=msq, in0=statsT[:, 2 + h:3 + h],
                                    scalar1=inv_n, scalar2=None, op0=ALU.mult)
            varp = small.tile([128, 1], f32, tag="varp")
            nc.vector.tensor_sub(out=varp, in0=msq, in1=musq)
            nc.vector.tensor_scalar_add(out=varp, in0=varp, scalar1=EPS)
            sd = small.tile([128, 1], f32, tag="sd")
            nc.scalar.activation(out=sd, in_=varp, func=AF.Sqrt)
            rsig = small.tile([128, 1], f32, tag="rsig")
            nc.vector.reciprocal(out=rsig, in_=sd)
            # one Newton step: rsig' = rsig * (1.5 - 0.5 * varp * rsig^2)
            t1 = small.tile([128, 1], f32, tag="t1")
            nc.vector.tensor_mul(out=t1, in0=rsig, in1=rsig)
            t2 = small.tile([128, 1], f32, tag="t2")
            nc.vector.tensor_mul(out=t2, in0=t1, in1=varp)
            nc.vector.tensor_scalar(out=t2, in0=t2, scalar1=-0.5, scalar2=1.5,
                                    op0=ALU.mult, op1=ALU.add)
            nc.vector.tensor_mul(out=rsig, in0=rsig, in1=t2)
            sc = small.tile([128, 1], f32, tag="sc")
            nc.vector.tensor_mul(out=sc, in0=rsig, in1=gamt_sb[h])
            bi = small.tile([128, 1], f32, tag="bi")
            nc.vector.tensor_mul(out=bi, in0=mu, in1=sc)
            nc.vector.tensor_sub(out=bi, in0=bett_sb[h], in1=bi)
            scale_h.append(sc)
            bias_h.append(bi)

        # ---------- phase F: BN apply + relu + residual (stays d-major) -----
        for h in range(2 if not TRUNC else 0):
            relu_sb = big.tile([128, LP], f32, tag=f"relu{h}", name=f"relu{h}")
            nc.scalar.activation(out=relu_sb, in_=tT_sb[h],
                                 func=AF.Relu, bias=bias_h[h], scale=scale_h[h])
            o_sb = big.tile([128, LP], f32, tag=f"o{h}", name=f"o{h}")
            nc.vector.tensor_add(out=o_sb, in0=relu_sb,
                                 in1=featT_sb[h].bitcast(f32))
            nc.sync.dma_start(out=out_d[128 * h:128 * (h + 1), :], in_=o_sb)

    nc.compile()
    return nc


def _get_nc(LP):
    if LP not in _NC_CACHE:
        _NC_CACHE[LP] = build_nc(LP)
    return _NC_CACHE[LP]


def kernel(**inputs):
    global LAST_RESULT
    feat = np.asarray(inputs["feat"], dtype=np.float32)
    bids = np.asarray(inputs["bids"])
    Wq = np.asarray(inputs["Wq"], dtype=np.float32)
    Wk = np.asarray(inputs["Wk"], dtype=np.float32)
    Wv = np.asarray(inputs["Wv"], dtype=np.float32)
    Wt = np.asarray(inputs["Wt"], dtype=np.float32)
    gamma = np.asarray(inputs["gamma"], dtype=np.float32)
    beta = np.asarray(inputs["beta"], dtype=np.float32)

    n, d = feat.shape
    assert d == D
    starts = np.searchsorted(bids, np.arange(NCORES)).astype(np.int64)
    ends = np.append(starts[1:], n)
    lens = (ends - starts).astype(np.int64)
    maxlen = int(lens.max())
    LP = max(LP_MIN, ((maxlen + 127) // 128) * 128)
    nc = _get_nc(LP)

    wqr = np.ascontiguousarray(np.concatenate([Wq] * 4, axis=1))
    wkr = np.ascontiguousarray(np.concatenate([Wk] * 4, axis=1))
    wv = np.ascontiguousarray(Wv)
    wt = Wt.astype(ml_dtypes.bfloat16)
    gamt = gamma.reshape(D, 1).copy()
    bett = beta.reshape(D, 1).copy()
    sel = np.zeros((4 * NCORES, 4), dtype=np.float32)
    for p in range(4 * NCORES):
        sel[p, p % 4] = 1.0

    in_maps = []
    for c in range(NCORES):
        seg = feat[starts[c]:ends[c]]
        L = seg.shape[0]
        featT = np.zeros((D, LP), dtype=np.float32)
        featT[:, :L] = seg.T
        maskf = np.zeros((LP, 1), dtype=np.float32)
        maskf[:L] = 1.0
        in_maps.append({
            "featT": featT, "maskf": maskf,
            "maskr": np.ascontiguousarray(maskf.reshape(1, LP)),
            "wqr": wqr, "wkr": wkr, "wv": wv, "wt": wt,
            "gamt": gamt, "bett": bett, "sel": sel,
        })

    trace_cores = None
    if os.environ.get("BASS_TRACE"):
        trace_cores = list(range(NCORES))
    res = bass_utils.run_bass_kernel_spmd(
        nc, in_maps, core_ids=list(range(NCORES)), trace_cores=trace_cores)
    LAST_RESULT = res

    out = np.empty((n, D), dtype=np.float32)
    for c in range(NCORES):
        out[starts[c]:ends[c]] = res.results[c]["out"].T[:lens[c]]
    return out


# revision 21
# speedup vs baseline: 1.6395x; 1.0836x over previous
"""Trainium2 Bass kernel for nn_ASVT_9500467658791 (ragged segment attention).

Pipeline (per point-cloud segment, one segment per NeuronCore, 8 cores):
  q/k/v = feat @ {Wq,Wk,Wv}  (1x1 convs)
  per-segment unscaled-softmax attention  r = softmax(q k^T) v
  t = r @ Wt ; BatchNorm over the full batch (training stats, synced across
  cores via a tiny AllGather) ; out = feat + relu(bn(t))

Layout strategy: everything d-major ("transposed") on chip.  The host
pre-transposes feat so no on-device transposes of the input are needed;
q^T/k^T are produced 4x-replicated across partition bands so the dqk=32
score matmuls can be packed 4-wide into the PE array with tile_position.
The q/k/score path runs in float32r (TF32-like).  Scores are computed
keys-major ([key, query]) and exp'd in kt-pair batches; the attention
matrix then streams as the MOVING operand against stationary v-halves,
accumulating r^T [d, q] directly (no transposes anywhere).  Softmax
denominators come from an extra mask-column matmul per key tile
([1, q] accumulator), inverted once per chunk and partition-broadcast.
BN stats reduce along the free axis of t^T; the final output stays
d-major (residual added from the exact f32 bits of featT) and the host
transposes it back during unsharding.
"""

import os
from contextlib import ExitStack

import numpy as np
import ml_dtypes

import concourse.bass as bass
import concourse.bacc as bacc
import concourse.tile as tile
from concourse import mybir
from concourse import bass_utils
from concourse.masks import make_identity

f32 = mybir.dt.float32
f32r = mybir.dt.float32r
bf16 = mybir.dt.bfloat16
AF = mybir.ActivationFunctionType
ALU = mybir.AluOpType
AX = mybir.AxisListType

NCORES = 8
D = 256
N_TOT = 16384
EPS = 1e-5
LP_MIN = 2176     # default segment pad (17 tiles); raised dynamically if needed
WARMUP_CC = bool(int(__import__("os").environ.get("WARMUP_CC", "1")))
WARMUP_MM = bool(int(__import__("os").environ.get("WARMUP_MM", "1")))
EXPPAIR = bool(int(__import__("os").environ.get("EXPPAIR", "1")))
STATS_TR = bool(int(__import__("os").environ.get("STATS_TR", "1")))
TRUNC = int(__import__("os").environ.get("TRUNC", "0"))
SINGLE_BANK_S = bool(int(__import__("os").environ.get("SINGLE_BANK_S", "1")))

LAST_RESULT = None  # BassKernelResults of the most recent run (for test harness)
_NC_CACHE = {}


def _chunks(LP):
    out, c0 = [], 0
    while c0 < LP:
        out.append((c0, min(512, LP - c0)))
        c0 += 512
    return out


def build_nc(LP):
    NT = LP // 128
    chunks = _chunks(LP)

    nc = bacc.Bacc("TRN2", target_bir_lowering=False, debug=False,
                   enable_asserts=True, num_devices=NCORES)

    featT_d = nc.dram_tensor("featT", [D, LP], f32r, kind="ExternalInput")
    maskf_d = nc.dram_tensor("maskf", [LP, 1], f32, kind="ExternalInput")
    maskr_d = nc.dram_tensor("maskr", [1, LP], f32, kind="ExternalInput")
    wqr_d = nc.dram_tensor("wqr", [D, 128], f32r, kind="ExternalInput")
    wkr_d = nc.dram_tensor("wkr", [D, 128], f32r, kind="ExternalInput")
    wv_d = nc.dram_tensor("wv", [D, D], f32r, kind="ExternalInput")
    wt_d = nc.dram_tensor("wt", [D, D], bf16, kind="ExternalInput")
    gamt_d = nc.dram_tensor("gamt", [D, 1], f32, kind="ExternalInput")
    bett_d = nc.dram_tensor("bett", [D, 1], f32, kind="ExternalInput")
    sel_d = nc.dram_tensor("sel", [4 * NCORES, 4], f32, kind="ExternalInput")
    out_d = nc.dram_tensor("out", [D, LP], f32, kind="ExternalOutput")

    cc_in = nc.dram_tensor("cc_in", [4, 128], f32, kind="Internal")
    cc_out = nc.dram_tensor("cc_out", [4 * NCORES, 128], f32, kind="Internal",
                            addr_space="Shared")
    cc_in2 = nc.dram_tensor("cc_in2", [4, 128], f32, kind="Internal")
    cc_out2 = nc.dram_tensor("cc_out2", [4 * NCORES, 128], f32, kind="Internal",
                             addr_space="Shared")
    dnd = nc.dram_tensor("dnd", [len(_chunks(LP)), 512], f32, kind="Internal")
    ccw_in = nc.dram_tensor("ccw_in", [1, 128], f32, kind="Internal")
    ccw_out = nc.dram_tensor("ccw_out", [NCORES, 128], f32, kind="Internal",
                             addr_space="Shared")

    with tile.TileContext(nc) as tc, ExitStack() as ctx:
        const = ctx.enter_context(tc.tile_pool(name="const", bufs=1))
        big = ctx.enter_context(tc.tile_pool(name="big", bufs=1))
        vpool = ctx.enter_context(tc.tile_pool(name="vpool", bufs=1))
        epool = ctx.enter_context(tc.tile_pool(name="epool", bufs=2))
        work = ctx.enter_context(tc.tile_pool(name="work", bufs=3))
        small = ctx.enter_context(tc.tile_pool(name="small", bufs=4))
        # PSUM: 8 banks = score-pairs 2x2 + rT accum 2 + denom 1 + tT 1
        psS = ctx.enter_context(tc.tile_pool(
            name="psS", bufs=(4 if SINGLE_BANK_S else 2), space="PSUM"))
        psV = ctx.enter_context(tc.tile_pool(name="psV", bufs=1, space="PSUM"))
        psD = ctx.enter_context(tc.tile_pool(name="psD", bufs=1, space="PSUM"))
        psA = ctx.enter_context(tc.tile_pool(name="psA", bufs=1, space="PSUM"))

        # ---------- PE clock warm-up: dense junk matmuls during input DMA ----
        ident = const.tile([128, 128], f32, tag="ident")
        make_identity(nc, ident)
        ident_b = const.tile([128, 128], bf16, tag="ident_b")
        nc.vector.tensor_copy(out=ident_b, in_=ident)
        if WARMUP_MM:
            ps_w = psA.tile([128, 512], f32, tag="a")
            for i in range(180):
                nc.tensor.matmul(ps_w[:, 0:128], lhsT=ident_b, rhs=ident_b,
                                 start=True, stop=True)
            warm_junk = const.tile([128, 1], f32, tag="warm_junk")
            nc.vector.tensor_copy(out=warm_junk, in_=ps_w[:, 0:1])

        # ---------- warm-up collective (runs on TOPSP during phase A) -------
        if WARMUP_CC:
            wz = const.tile([1, 128], f32, tag="wz")
            nc.vector.memset(wz, 0.0)
            nc.sync.dma_start(out=ccw_in[:, :], in_=wz)
            nc.gpsimd.collective_compute(
                "AllGather", ALU.bypass, replica_groups=[list(range(NCORES))],
                ins=[ccw_in[:, :]], outs=[ccw_out[:, :]])

        # ---------- constants / inputs ----------
        featT_sb = [big.tile([128, LP], f32r, tag=f"featT{h}", name=f"featT{h}")
                    for h in range(2)]
        for h in range(2):
            for (c0, cw) in chunks:
                nc.sync.dma_start(out=featT_sb[h][:, c0:c0 + cw],
                                  in_=featT_d[128 * h:128 * (h + 1), c0:c0 + cw])

        maskf_sb = const.tile([128, NT], f32, tag="maskf")
        nc.sync.dma_start(out=maskf_sb,
                          in_=maskf_d.rearrange("(n p) one -> p (n one)", p=128))
        maskb_sb = const.tile([128, NT], bf16, tag="maskb")
        nc.vector.tensor_copy(out=maskb_sb, in_=maskf_sb)
        maskbc_sb = const.tile([128, LP], f32, tag="maskbc")
        _mr = maskr_d[0:1, :]
        nc.sync.dma_start(out=maskbc_sb, in_=bass.AP(
            tensor=_mr.tensor, offset=_mr.offset, ap=[[0, 128]] + list(_mr.ap[1:])))
        ones128 = const.tile([128, 128], bf16, tag="ones128")
        nc.vector.memset(ones128, 1.0)
        msk128 = const.tile([128, 128], bf16, tag="msk128")
        _mb = maskb_sb[:, NT - 1:NT]
        nc.vector.tensor_copy(out=msk128, in_=bass.AP(
            tensor=_mb.tensor, offset=_mb.offset, ap=[list(_mb.ap[0]), [0, 128]]))

        wqr_sb = [const.tile([128, 128], f32r, tag=f"wqr{h}", name=f"wqr{h}")
                  for h in range(2)]
        wkr_sb = [const.tile([128, 128], f32r, tag=f"wkr{h}", name=f"wkr{h}")
                  for h in range(2)]
        wv_sb = [const.tile([128, D], f32r, tag=f"wv{h}", name=f"wv{h}")
                 for h in range(2)]
        wt_sb = [const.tile([128, D], bf16, tag=f"wt{h}", name=f"wt{h}")
                 for h in range(2)]
        for h in range(2):
            sl = slice(128 * h, 128 * (h + 1))
            nc.sync.dma_start(out=wqr_sb[h], in_=wqr_d[sl, :])
            nc.sync.dma_start(out=wkr_sb[h], in_=wkr_d[sl, :])
            nc.sync.dma_start(out=wv_sb[h], in_=wv_d[sl, :])
            nc.sync.dma_start(out=wt_sb[h], in_=wt_d[sl, :])
        gamt_sb = [const.tile([128, 1], f32, tag=f"gam{h}", name=f"gam{h}")
                   for h in range(2)]
        bett_sb = [const.tile([128, 1], f32, tag=f"bet{h}", name=f"bet{h}")
                   for h in range(2)]
        for h in range(2):
            sl = slice(128 * h, 128 * (h + 1))
            nc.sync.dma_start(out=gamt_sb[h], in_=gamt_d[sl, :])
            nc.sync.dma_start(out=bett_sb[h], in_=bett_d[sl, :])
        sel_sb = const.tile([4 * NCORES, 4], f32, tag="sel")
        nc.sync.dma_start(out=sel_sb, in_=sel_d[:, :])

        # ---------- phase A: projections ----------
        # qT_rep / kT_rep [128, LP]: each 32-row band holds the full
        # [32, LP] q^T / k^T (host replicated W 4x along columns).
        qT_sb = big.tile([128, LP], f32r, tag="qT")
        kT_sb = big.tile([128, LP], f32r, tag="kT")
        for (c0, cw) in chunks:
            csl = slice(c0, c0 + cw)
            for wrep, dst in ((wqr_sb, qT_sb), (wkr_sb, kT_sb)):
                ps = psS.tile([128, 512 if SINGLE_BANK_S else 1024], f32, tag="s")
                nc.tensor.matmul(ps[:, :cw], lhsT=wrep[0], rhs=featT_sb[0][:, csl],
                                 start=True, stop=False)
                nc.tensor.matmul(ps[:, :cw], lhsT=wrep[1], rhs=featT_sb[1][:, csl],
                                 start=False, stop=True)
                nc.vector.tensor_copy(out=dst[:, csl], in_=ps[:, :cw])

        v_sb = []
        for i in range(NT):
            isl = slice(128 * i, 128 * (i + 1))
            ps = psS.tile([128, 512], f32, tag="s", name=f"psv_proj{i}")
            nc.tensor.matmul(ps[:, 0:D], lhsT=featT_sb[0][:, isl], rhs=wv_sb[0],
                             start=True, stop=False)
            nc.tensor.matmul(ps[:, 0:D], lhsT=featT_sb[1][:, isl], rhs=wv_sb[1],
                             start=False, stop=True)
            vt = vpool.tile([128, D], bf16, tag=f"v{i}", name=f"v{i}")
            nc.vector.tensor_copy(out=vt, in_=ps[:, 0:D])
            v_sb.append(vt)

        rT_sb = [big.tile([128, LP], bf16, tag=f"rT{h}", name=f"rT{h}")
                 for h in range(2)]
        tT_sb = [big.tile([128, LP], f32, tag=f"tT{h}", name=f"tT{h}")
                 for h in range(2)]
        nch = len(chunks)
        sums_t = [const.tile([128, nch], f32, tag=f"st{h}", name=f"st{h}")
                  for h in range(2)]
        sums_q = [const.tile([128, nch], f32, tag=f"sq{h}", name=f"sq{h}")
                  for h in range(2)]

        # ---------- phases B-D: attention + r^T + t^T, chunked over queries --
        for ci, (c0, cw) in (enumerate(chunks) if TRUNC != 2 else []):
            csl = slice(c0, c0 + cw)
            pairs = [(kt, kt + 1 if kt + 1 < NT else None)
                     for kt in range(0, NT, 2)]

            ps_rt = None if TRUNC == 4 else [
                psV.tile([128, 512], f32, tag=f"v{h}", name=f"psv{h}")
                for h in range(2)]
            ps_d = None if TRUNC == 4 else psD.tile([128, 512], f32, tag="d")

            exp_of = {}
            for pi, (ka, kb) in enumerate(pairs):
                if SINGLE_BANK_S:
                    pss = [psS.tile([128, 512], f32, tag="s", name=f"pss{s_}")
                           for s_ in range(2 if kb is not None else 1)]
                else:
                    ps = psS.tile([128, 1024], f32, tag="s")
                for sub, kt in enumerate([ka] + ([kb] if kb is not None else [])):
                    i = (2 * pi + sub) % 4
                    ksl = slice(128 * kt, 128 * (kt + 1))
                    bsl = slice(32 * i, 32 * (i + 1))
                    dst_ap = (pss[sub][:, :cw] if SINGLE_BANK_S
                              else ps[:, sub * cw:sub * cw + cw])
                    nc.tensor.matmul(dst_ap,
                                     lhsT=kT_sb[bsl, ksl], rhs=qT_sb[bsl, csl],
                                     start=True, stop=True,
                                     tile_position=(32 * i, 0))
                et = epool.tile([128, 1024], bf16, tag=f"e{pi}", name=f"e{pi}")
                nsub = 1 if kb is None else 2
                if EXPPAIR and not SINGLE_BANK_S:
                    ew = cw if kb is None else 2 * cw
                    nc.scalar.activation(out=et[:, :ew], in_=ps[:, :ew], func=AF.Exp)
                else:
                    for s_ in range(nsub):
                        src_ap = (pss[s_][:, :cw] if SINGLE_BANK_S
                                  else ps[:, s_ * cw:s_ * cw + cw])
                        nc.scalar.activation(out=et[:, s_ * cw:s_ * cw + cw],
                                             in_=src_ap, func=AF.Exp)
                for sub, kt in enumerate([ka] + ([kb] if kb is not None else [])):
                    exp_of[kt] = (et, sub)

                # attn@v for the pair right away: rT += v[kt]^T-stationary MMs
                for sub, kt in ([] if TRUNC == 4 else
                                list(enumerate([ka] + ([kb] if kb is not None else [])))):
                    et_, s_ = exp_of[kt]
                    esl = slice(s_ * cw, s_ * cw + cw)
                    first = kt == 0
                    last = kt == NT - 1
                    for h in range(2):
                        nc.tensor.matmul(
                            ps_rt[h][:, :cw], lhsT=v_sb[kt][:, 128 * h:128 * (h + 1)],
                            rhs=et_[:, esl], start=first, stop=last)

            if TRUNC != 4:
                for kt in range(NT):
                    et_, s_ = exp_of[kt]
                    nc.tensor.matmul(ps_d[:, :cw],
                                     lhsT=(msk128 if kt == NT - 1 else ones128),
                                     rhs=et_[:, s_ * cw:s_ * cw + cw],
                                     start=(kt == 0), stop=(kt == NT - 1))
            if TRUNC == 4:
                for h in range(2):
                    ej = exp_of[NT - 1][0]
                    nc.vector.tensor_copy(out=rT_sb[h][:, csl],
                                          in_=ej[:, 0:cw])
            if TRUNC == 3:
                for h in range(2):
                    nc.vector.tensor_copy(out=rT_sb[h][:, csl],
                                          in_=ps_rt[h][:, :cw])
                djunk = small.tile([128, 1], f32, tag="djunk")
                nc.vector.tensor_copy(out=djunk, in_=ps_d[:, 0:1])
            # softmax denominators -> masked reciprocal (rows identical)
            if TRUNC not in (3, 4):
                dnf = work.tile([128, 512], f32, tag="dnf")
                nc.vector.tensor_scalar_max(out=dnf[:, :cw], in0=ps_d[:, :cw],
                                            scalar1=1e-30)
                rec = work.tile([128, 512], f32, tag="recd")
                nc.vector.reciprocal_approx_fast(out=rec[:, :cw], in_=dnf[:, :cw])
                nc.vector.tensor_mul(out=rec[:, :cw], in0=rec[:, :cw],
                                     in1=maskbc_sb[:, csl])
                for h in range(2):
                    nc.vector.tensor_mul(out=rT_sb[h][:, csl],
                                         in0=ps_rt[h][:, :cw],
                                         in1=rec[:, :cw])

            # tT = Wt^T @ rT for this query chunk + BN partial stats
            for h in range(2):
                hsl = slice(128 * h, 128 * (h + 1))
                ps_t = psA.tile([128, 512], f32, tag="a")
                nc.tensor.matmul(ps_t[:, :cw], lhsT=wt_sb[0][:, hsl],
                                 rhs=rT_sb[0][:, csl], start=True, stop=False)
                nc.tensor.matmul(ps_t[:, :cw], lhsT=wt_sb[1][:, hsl],
                                 rhs=rT_sb[1][:, csl], start=False, stop=True)
                nc.scalar.activation(out=tT_sb[h][:, csl], in_=ps_t[:, :cw],
                                     func=AF.Copy,
                                     accum_out=sums_t[h][:, ci:ci + 1])
                sq = work.tile([128, 512], f32, tag="sq")
                nc.vector.tensor_mul(out=sq[:, :cw], in0=tT_sb[h][:, csl],
                                     in1=tT_sb[h][:, csl])
                nc.vector.reduce_sum(out=sums_q[h][:, ci:ci + 1], in_=sq[:, :cw],
                                     axis=AX.X)

            if ci == nch - 3 and not TRUNC:
                # early partial-stats AllGather (chunks 0..nch-2): overlaps the
                # last chunk's compute and re-syncs core skew before AG2
                stf1 = const.tile([128, 4], f32, tag="stf1")
                for h in range(2):
                    nc.vector.reduce_sum(out=stf1[:, h:h + 1],
                                         in_=sums_t[h][:, 0:nch - 2], axis=AX.X)
                    nc.vector.reduce_sum(out=stf1[:, 2 + h:3 + h],
                                         in_=sums_q[h][:, 0:nch - 2], axis=AX.X)
                ps_st1 = psD.tile([4, 128], f32, tag="d", name="ps_st1")
                nc.tensor.transpose(ps_st1, stf1, ident)
                stp1 = const.tile([4, 128], f32, tag="stp1")
                nc.vector.tensor_copy(out=stp1, in_=ps_st1)
                nc.sync.dma_start(out=cc_in[:, :], in_=stp1)
                nc.gpsimd.collective_compute(
                    "AllGather", ALU.bypass,
                    replica_groups=[list(range(NCORES))],
                    ins=[cc_in[:, :]], outs=[cc_out[:, :]])

        if TRUNC in (1, 3, 4):
            for h in range(2):
                nc.sync.dma_start(out=out_d[128 * h:128 * (h + 1), :], in_=tT_sb[h])
        elif TRUNC >= 2:
            for h, src_t in ((0, qT_sb), (1, kT_sb)):
                nc.sync.dma_start(out=out_d[128 * h:128 * (h + 1), :],
                                  in_=src_t.bitcast(f32))
        # ---------- phase E: global BN stats via AllGather ----------
        stf = None if TRUNC else const.tile([128, 4], f32, tag="stf")
        for h in range(2 if not TRUNC else 0):
            nc.vector.reduce_sum(out=stf[:, h:h + 1],
                                  in_=sums_t[h][:, nch - 2:nch], axis=AX.X)
            nc.vector.reduce_sum(out=stf[:, 2 + h:3 + h],
                                  in_=sums_q[h][:, nch - 2:nch], axis=AX.X)
        if not TRUNC:
            ps_st = psD.tile([4, 128], f32, tag="d", name="ps_st")
            nc.tensor.transpose(ps_st, stf, ident)
            stp = const.tile([4, 128], f32, tag="stp")
            nc.vector.tensor_copy(out=stp, in_=ps_st)
            nc.sync.dma_start(out=cc_in2[:, :], in_=stp)
        if not TRUNC:
            nc.gpsimd.collective_compute(
                "AllGather", ALU.bypass,
                replica_groups=[list(range(NCORES))],
                ins=[cc_in2[:, :]], outs=[cc_out2[:, :]])
            ag_sb = const.tile([4 * NCORES, 128], f32, tag="ag")
            nc.sync.dma_start(out=ag_sb, in_=cc_out[:, :])
            ag2_sb = const.tile([4 * NCORES, 128], f32, tag="ag2")
            nc.sync.dma_start(out=ag2_sb, in_=cc_out2[:, :])
            ps_g = psA.tile([128, 512], f32, tag="a", name="ps_g")
            nc.tensor.matmul(ps_g[:, 0:4], lhsT=ag_sb, rhs=sel_sb,
                             start=True, stop=False)
            nc.tensor.matmul(ps_g[:, 0:4], lhsT=ag2_sb, rhs=sel_sb,
                             start=False, stop=True)
            statsT = const.tile([128, 4], f32, tag="statsT")
            nc.vector.tensor_copy(out=statsT, in_=ps_g[:, 0:4])

        scale_h, bias_h = [], []
        inv_n = 1.0 / float(N_TOT)
        for h in range(2 if not TRUNC else 0):
            mu = small.tile([128, 1], f32, tag="mu")
            nc.vector.tensor_scalar_mul(out=mu, in0=statsT[:, h:h + 1], scalar1=inv_n)
            musq = small.tile([128, 1], f32, tag="musq")
            nc.vector.tensor_mul(out=musq, in0=mu, in1=mu)
            msq = small.tile([128, 1], f32, tag="msq")
            nc.vector.tensor_scalar(out=msq, in0=statsT[:, 2 + h:3 + h],
                                    scalar1=inv_n, scalar2=None, op0=ALU.mult)
            varp = small.tile([128, 1], f32, tag="varp")
            nc.vector.tensor_sub(out=varp, in0=msq, in1=musq)
            nc.vector.tensor_scalar_add(out=varp, in0=varp, scalar1=EPS)
            sd = small.tile([128, 1], f32, tag="sd")
            nc.scalar.activation(out=sd, in_=varp, func=AF.Sqrt)
            rsig = small.tile([128, 1], f32, tag="rsig")
            nc.vector.reciprocal(out=rsig, in_=sd)
            # one Newton step: rsig' = rsig * (1.5 - 0.5 * varp * rsig^2)
            t1 = small.tile([128, 1], f32, tag="t1")
            nc.vector.tensor_mul(out=t1, in0=rsig, in1=rsig)
            t2 = small.tile([128, 1], f32, tag="t2")
            nc.vector.tensor_mul(out=t2, in0=t1, in1=varp)
            nc.vector.tensor_scalar(out=t2, in0=t2, scalar1=-0.5, scalar2=1.5,
                                    op0=ALU.mult, op1=ALU.add)
            nc.vector.tensor_mul(out=rsig, in0=rsig, in1=t2)
            sc = small.tile([128, 1], f32, tag="sc")
            nc.vector.tensor_mul(out=sc, in0=rsig, in1=gamt_sb[h])
            bi = small.tile([128, 1], f32, tag="bi")
            nc.vector.tensor_mul(out=bi, in0=mu, in1=sc)
            nc.vector.tensor_sub(out=bi, in0=bett_sb[h], in1=bi)
            scale_h.append(sc)
            bias_h.append(bi)

        # ---------- phase F: BN apply + relu + residual (stays d-major) -----
        for h in range(2 if not TRUNC else 0):
            relu_sb = big.tile([128, LP], f32, tag=f"relu{h}", name=f"relu{h}")
            nc.scalar.activation(out=relu_sb, in_=tT_sb[h],
                                 func=AF.Relu, bias=bias_h[h], scale=scale_h[h])
            o_sb = big.tile([128, LP], f32, tag=f"o{h}", name=f"o{h}")
            nc.vector.tensor_add(out=o_sb, in0=relu_sb,
                                 in1=featT_sb[h].bitcast(f32))
            nc.sync.dma_start(out=out_d[128 * h:128 * (h + 1), :], in_=o_sb)

    nc.compile()
    return nc


def _get_nc(LP):
    if LP not in _NC_CACHE:
        _NC_CACHE[LP] = build_nc(LP)
    return _NC_CACHE[LP]


def kernel(**inputs):
    global LAST_RESULT
    feat = np.asarray(inputs["feat"], dtype=np.float32)
    bids = np.asarray(inputs["bids"])
    Wq = np.asarray(inputs["Wq"], dtype=np.float32)
    Wk = np.asarray(inputs["Wk"], dtype=np.float32)
    Wv = np.asarray(inputs["Wv"], dtype=np.float32)
    Wt = np.asarray(inputs["Wt"], dtype=np.float32)
    gamma = np.asarray(inputs["gamma"], dtype=np.float32)
    beta = np.asarray(inputs["beta"], dtype=np.float32)

    n, d = feat.shape
    assert d == D
    starts = np.searchsorted(bids, np.arange(NCORES)).astype(np.int64)
    ends = np.append(starts[1:], n)
    lens = (ends - starts).astype(np.int64)
    maxlen = int(lens.max())
    LP = max(LP_MIN, ((maxlen + 127) // 128) * 128)
    nc = _get_nc(LP)

    wqr = np.ascontiguousarray(np.concatenate([Wq] * 4, axis=1))
    wkr = np.ascontiguousarray(np.concatenate([Wk] * 4, axis=1))
    wv = np.ascontiguousarray(Wv)
    wt = Wt.astype(ml_dtypes.bfloat16)
    gamt = gamma.reshape(D, 1).copy()
    bett = beta.reshape(D, 1).copy()
    sel = np.zeros((4 * NCORES, 4), dtype=np.float32)
    for p in range(4 * NCORES):
        sel[p, p % 4] = 1.0

    in_maps = []
    for c in range(NCORES):
        seg = feat[starts[c]:ends[c]]
        L = seg.shape[0]
        featT = np.zeros((D, LP), dtype=np.float32)
        featT[:, :L] = seg.T
        maskf = np.zeros((LP, 1), dtype=np.float32)
        maskf[:L] = 1.0
        in_maps.append({
            "featT": featT, "maskf": maskf,
            "maskr": np.ascontiguousarray(maskf.reshape(1, LP)),
            "wqr": wqr, "wkr": wkr, "wv": wv, "wt": wt,
            "gamt": gamt, "bett": bett, "sel": sel,
        })

    trace_cores = None
    if os.environ.get("BASS_TRACE"):
        trace_cores = list(range(NCORES))
    res = bass_utils.run_bass_kernel_spmd(
        nc, in_maps, core_ids=list(range(NCORES)), trace_cores=trace_cores)
    LAST_RESULT = res

    out = np.empty((n, D), dtype=np.float32)
    for c in range(NCORES):
        out[starts[c]:ends[c]] = res.results[c]["out"].T[:lens[c]]
    return out
